# revision 30
# baseline (speedup 1.0000x reference)
"""Trainium2 Bass kernel for nn_Attention (sparse_attention variant).

Computes, for inputs hidden/encoder_outputs [B,S,D], c_t [B,D], W [OUT,3D],
b [OUT], v [OUT]:

    cat       = concat([hidden, broadcast(c_t), encoder_outputs], axis=2)
    energy    = relu(cat @ W.T + b)            # [B, S, OUT]
    attention = energy @ v                     # [B, S]
    out       = softmax(attention, axis=1)

Strategy (8 NeuronCores, data-parallel over batch, 2 batches/core):
  - Split W = [W1 | W2 | W3] over the feature axis.
  - Matmuls run in fp16 (fp32 PSUM accumulate).  The contraction dim f
    must sit on SBUF partitions, so X feeds in transposed.  The first
    4 s-tiles are loaded naturally (fp32, scalar ring, from t=0), DVE
    cast, and transposed on the PE while W streams in -- this lets the
    main loop start ~50us in instead of ~98us.  The remaining 28
    s-tiles go the proven scratch route: SWDGE DRAM->DRAM fp16 casts
    (gated on the first half of W so W wins the HBM race), then HWDGE
    xbar-transpose loads [f, s] on the sync ring.
  - W: fp32 half-row loads alternating sync/scalar rings, rows 0..511
    first; fp16-cast on DVE; transposed on the PE.  Bank-0 matmuls of
    s-tiles 0..2 plus the bank-0 half of c2 run as soon as W rows
    0..511 are on-chip (phase split), hiding the tail of the W load.
  - c2[b,:] = c_t[b] @ W2.T + b is computed per batch with [1,512]
    PSUM rows (batch 1 lazily at s-tile 12) and broadcast via a
    ones-column matmul.
  - Main loop per 128-row s-tile: accumulate pre[s, o] over 16 f-chunks
    x 2 PSUM banks; VectorE adds the broadcast c2 row and does a fused
    relu(pre)*v + row-sum (accum_out) -> attention logits.
  - Softmax over S=2048 per batch: 128x16 tile, DVE free-dim reduce +
    GpSimd partition all-reduce, ScalarE exp, DVE normalize.
"""

import sys
import numpy as np

for _p in ("/opt/trn_rl_repo",):
    if _p not in sys.path:
        sys.path.insert(0, _p)

import concourse.bass as bass
import concourse.bacc as bacc
import concourse.tile as tile
from concourse.tile import add_dep_helper
from concourse import mybir, bass_isa
from concourse.bass_utils import run_bass_kernel_spmd
from concourse.masks import make_identity

F32 = mybir.dt.float32
F16 = mybir.dt.float16
BF16 = mybir.dt.bfloat16
AF = mybir.ActivationFunctionType
ALU = mybir.AluOpType

B, S, D, OUT = 16, 2048, 1024, 1024
N_CORES = 8
B_LOC = B // N_CORES            # batches per core
S_LOC = B_LOC * S               # 4096 rows of X per core
N_ST = S_LOC // 128             # 32 s-tiles per core
ST_PER_B = S // 128             # 16 s-tiles per batch
FC = D // 128                   # 8 feature chunks per tensor
NB = OUT // 512                 # 2 PSUM banks across OUT
PHASE_TILES = 2                 # s-tiles that run bank0 before W fully loaded
NAT = 2                         # s-tiles via natural load + PE transpose
SB_ROWS = 1024                  # scratch s-block rows
N_SB = 4                        # scratch blocks cover st4..31


def build_nc():
    nc = bacc.Bacc("TRN2", target_bir_lowering=False, debug=False,
                   num_devices=N_CORES, dynamic_dma_scratch_size=32768)

    hid = nc.dram_tensor("hidden", [S_LOC, D], F32, kind="ExternalInput").ap()
    enc = nc.dram_tensor("enc", [S_LOC, D], F32, kind="ExternalInput").ap()
    ct = nc.dram_tensor("ct", [B_LOC, D], F32, kind="ExternalInput").ap()
    Wd = nc.dram_tensor("W", [OUT, 3 * D], F32, kind="ExternalInput").ap()
    bd = nc.dram_tensor("b", [OUT], F32, kind="ExternalInput").ap()
    vd = nc.dram_tensor("v", [OUT], F32, kind="ExternalInput").ap()
    outd = nc.dram_tensor("out", [B_LOC, S], F32, kind="ExternalOutput").ap()
    # scratch blocks: rows 256.., 1280.., 2304.., 3328.. (last = 768 rows)
    SBR = [1024, 1024, 1024, 768]
    SB0 = [256, 1280, 2304, 3328]
    scr_h = [nc.dram_tensor(f"scr_h{i}", [SBR[i], D], F16).ap()
             for i in range(N_SB)]
    scr_e = [nc.dram_tensor(f"scr_e{i}", [SBR[i], D], F16).ap()
             for i in range(N_SB)]

    with tile.TileContext(nc) as tc:
        with (
            tc.tile_pool(name="const", bufs=1) as cpool,
            tc.tile_pool(name="wT", bufs=1) as wpool,
            tc.tile_pool(name="wload", bufs=2) as wload,
            tc.tile_pool(name="xnat", bufs=1) as xnat,
            tc.tile_pool(name="xT", bufs=1) as xTp,
            tc.tile_pool(name="scratch", bufs=1) as spool,
            tc.tile_pool(name="sm", bufs=2) as smpool,
            tc.tile_pool(name="eps", bufs=2, space=bass.MemorySpace.PSUM) as eps,
            tc.tile_pool(name="cps", bufs=1, space=bass.MemorySpace.PSUM) as cps,
            tc.tile_pool(name="ptp", bufs=2, space=bass.MemorySpace.PSUM) as ptp,
        ):
            ident = cpool.tile([128, 128], F16)
            make_identity(nc, ident[:])
            ones_k1 = cpool.tile([1, 128], F16)
            nc.vector.memset(ones_k1[:], 1.0)
            att_all = cpool.tile([128, N_ST], F32)   # attention logits

            # ---- natural loads for the first NAT s-tiles (scalar ring) ----
            x32s = []
            for st in range(NAT):
                rows = slice(st * 128, (st + 1) * 128)
                x32h = xnat.tile([128, D], F32, tag="x32", bufs=2)
                nc.scalar.dma_start(x32h[:], hid[rows, :])
                x32e = xnat.tile([128, D], F32, tag="x32e", bufs=2)
                nc.scalar.dma_start(x32e[:], enc[rows, :])
                x32s.append((x32h, x32e))

            # ---- small constants ------------------------------------------
            v_h = cpool.tile([1, OUT], F16)
            nc.gpsimd.dma_start(v_h[:], vd[None, :])
            b_f = cpool.tile([1, OUT], F16)
            nc.gpsimd.dma_start(b_f[:], bd[None, :])
            ctT_f = cpool.tile([128, FC, B_LOC], F32)
            for bb in range(B_LOC):
                nc.sync.dma_start(ctT_f[:, :, bb],
                                  ct[bb].rearrange("(fc p) -> p fc", p=128))
            ctT_h = cpool.tile([128, FC, B_LOC], F16)
            nc.vector.tensor_copy(ctT_h[:], ctT_f[:])

            # ---- W: fp32 half-row loads, rows 0..511 first ----------------
            # wT[p, j, o] = W[o, j*128+p]; j in [0,8)=W1, [8,16)=W2,
            # [16,24)=W3.
            wT = wpool.tile([128, 3 * FC, OUT], F16)
            HALF_F = 3 * D // 2
            w_dmas = []
            w16s = {}

            def load_w(oc):
                w16 = []
                for half in range(2):
                    w_nat = wload.tile([128, HALF_F], F32, tag="wnat", bufs=2)
                    w_eng = nc.sync if (2 * oc + half) % 2 == 0 else nc.scalar
                    w_dma = w_eng.dma_start(
                        w_nat[:], Wd[oc * 128:(oc + 1) * 128,
                                     half * HALF_F:(half + 1) * HALF_F])
                    w_dmas.append(w_dma)
                    w_s = wload.tile([128, HALF_F], F16, tag="ws", bufs=6)
                    nc.vector.tensor_copy(w_s[:], w_nat[:])
                    w16.append(w_s)
                w16s[oc] = w16

            def pe_transpose_w(oc, jgs):
                for j0 in jgs:
                    half = j0 // 12
                    pt = ptp.tile([128, 4, 128], F16, tag="tp")
                    for k in range(4):
                        j = j0 + k - half * 12
                        nc.tensor.transpose(
                            pt[:, k, :],
                            w16s[oc][half][:, j * 128:(j + 1) * 128],
                            ident[:])
                    nc.vector.tensor_copy(
                        wT[:, j0:j0 + 4, oc * 128:(oc + 1) * 128], pt[:])

            for oc in range(4):
                load_w(oc)

            # natural-X casts + PE transposes for st0..NAT-1, interleaved
            # with the W transposes of rows 0..511
            def pe_transpose_x(st):
                x16h = xnat.tile([128, D], F16, tag="x16", bufs=1)
                nc.vector.tensor_copy(x16h[:], x32s[st][0][:])
                x16e = xnat.tile([128, D], F16, tag="x16e", bufs=1)
                nc.vector.tensor_copy(x16e[:], x32s[st][1][:])
                xTh = xTp.tile([128, FC, 128], F16, tag="xTn", bufs=2)
                xTe = xTp.tile([128, FC, 128], F16, tag="xTne", bufs=2)
                for src, dst in ((x16h, xTh), (x16e, xTe)):
                    for j0 in (0, 4):
                        pt = ptp.tile([128, 4, 128], F16, tag="tp")
                        for k in range(4):
                            j = j0 + k
                            nc.tensor.transpose(
                                pt[:, k, :], src[:, j * 128:(j + 1) * 128],
                                ident[:])
                        nc.vector.tensor_copy(dst[:, j0:j0 + 4, :], pt[:])
                return (xTh, xTe, 0)

            xts = {}
            for st in range(NAT):
                xts[st] = pe_transpose_x(st)
            for oc in range(4):
                pe_transpose_w(oc, (0, 4, 8, 12, 16, 20))

            # ---- X scratch casts: block 0 ungated, rest yield to W --------
            gate = w_dmas[7]
            for i in range(N_SB):
                ch = nc.gpsimd.dma_start(
                    scr_h[i][:], hid[SB0[i]:SB0[i] + SBR[i], :])
                ce = nc.gpsimd.dma_start(
                    scr_e[i][:], enc[SB0[i]:SB0[i] + SBR[i], :])
                if i > 0:
                    add_dep_helper(ch.ins, gate.ins,
                                   reason="X casts yield HBM to W loads")
                    add_dep_helper(ce.ins, gate.ins,
                                   reason="X casts yield HBM to W loads")

            # rest of W: h0 halves land first (sync); casts follow in order
            for oc in range(4, 8):
                load_w(oc)

            # ---- scratch-block xbar loads (sync ring) ---------------------
            def emit_xblock(i, xts):
                xTh = xTp.tile([128, FC, SB_ROWS], F16, tag="xTh", bufs=2)
                xTe = xTp.tile([128, FC, SB_ROWS], F16, tag="xTe", bufs=2)
                r = SBR[i]
                for fc in range(FC):
                    nc.sync.dma_start(xTh[:, fc, :r],
                                      scr_h[i][:, fc * 128:(fc + 1) * 128],
                                      transpose=True)
                    nc.sync.dma_start(xTe[:, fc, :r],
                                      scr_e[i][:, fc * 128:(fc + 1) * 128],
                                      transpose=True)
                for k in range(r // 128):
                    xts[NAT + i * 8 + k] = (xTh, xTe, k * 128)

            def emit_mm(e_ps, ob, st):
                xTh, xTe, off = xts[st]
                ssl = slice(off, off + 128)
                sl = slice(ob * 512, (ob + 1) * 512)
                for fc in range(FC):
                    nc.tensor.matmul(e_ps[:, sl], xTh[:, fc, ssl],
                                     wT[:, fc, sl],
                                     start=(fc == 0), stop=False)
                for fc in range(FC):
                    nc.tensor.matmul(e_ps[:, sl], xTe[:, fc, ssl],
                                     wT[:, 2 * FC + fc, sl],
                                     start=False, stop=(fc == FC - 1))

            # vbc[p, o] = v[o] via PSUM half-tiles
            vbc = cpool.tile([128, OUT], F16)
            for ob in range(NB):
                sl = slice(ob * 512, (ob + 1) * 512)
                vps = cps.tile([128, 512], F32, tag="cps")
                nc.tensor.matmul(vps[:], ones_k1[:], v_h[:, sl],
                                 start=True, stop=True)
                nc.vector.tensor_copy(vbc[:, sl], vps[:])

            # c2[b, o] = c_t[b] @ W2.T + b via PSUM half-tiles
            c2b_sb = {}
            c2bc_sb = {}

            def emit_c2_bank(bb, ob):
                sl = slice(ob * 512, (ob + 1) * 512)
                cp = cps.tile([1, 512], F32, tag="cps")
                for fc in range(FC):
                    nc.tensor.matmul(cp[:], ctT_h[:, fc, bb:bb + 1],
                                     wT[:, FC + fc, sl],
                                     start=(fc == 0), stop=(fc == FC - 1))
                if bb not in c2b_sb:
                    c2b_t = cpool.tile([1, OUT], F16, tag=f"c2b_{bb}")
                    c2b_sb[bb] = c2b_t
                nc.vector.tensor_add(c2b_sb[bb][:, sl], cp[:], b_f[:, sl])

            def emit_c2bc(bb):
                if bb not in c2bc_sb:
                    c2bc_t = cpool.tile([128, OUT], F16, tag=f"c2bc_{bb}")
                    c2bc_sb[bb] = c2bc_t
                for ob in range(NB):
                    sl = slice(ob * 512, (ob + 1) * 512)
                    cp = cps.tile([128, 512], F32, tag="cps")
                    nc.tensor.matmul(cp[:], ones_k1[:], c2b_sb[bb][:, sl],
                                     start=True, stop=True)
                    nc.vector.tensor_copy(c2bc_sb[bb][:, sl], cp[:])

            # --- phase A: bank 0 only (W rows 0..511 suffice) --------------
            emit_c2_bank(0, 0)
            e_tiles = {}
            for st in range(PHASE_TILES):
                e_ps = eps.tile([128, OUT], F32, tag="eps")
                e_tiles[st] = e_ps
                emit_mm(e_ps, 0, st)

            # W transposes for rows 512..1023 (PE, overlapped with phase A);
            # half-0 j-groups first: their casts land well before half-1's
            for oc in range(4, 8):
                pe_transpose_w(oc, (0, 4, 8))
            # first scratch block's loads go out on sync as soon as the
            # (ungated) block-0 casts land
            emit_xblock(0, xts)
            for oc in range(4, 8):
                pe_transpose_w(oc, (12, 16, 20))

            # --- phase B ---------------------------------------------------
            emit_c2_bank(0, 1)
            emit_c2bc(0)

            def emit_softmax(bb):
                sl = slice(bb * ST_PER_B, (bb + 1) * ST_PER_B)
                m1 = smpool.tile([128, 1], F32, tag="m1")
                nc.vector.tensor_reduce(m1[:], att_all[:, sl],
                                        axis=mybir.AxisListType.X,
                                        op=ALU.max)
                mall = smpool.tile([128, 1], F32, tag="mall")
                nc.gpsimd.partition_all_reduce(mall[:], m1[:], channels=128,
                                               reduce_op=bass_isa.ReduceOp.max)
                nmall = smpool.tile([128, 1], F32, tag="nmall")
                nc.vector.tensor_scalar_mul(nmall[:], mall[:], -1.0)
                ex = smpool.tile([128, ST_PER_B], F32, tag="ex")
                rs = smpool.tile([128, 1], F32, tag="rs")
                nc.scalar.activation(ex[:], att_all[:, sl], AF.Exp,
                                     bias=nmall[:], accum_out=rs[:])
                tot = smpool.tile([128, 1], F32, tag="tot")
                nc.gpsimd.partition_all_reduce(tot[:], rs[:], channels=128,
                                               reduce_op=bass_isa.ReduceOp.add)
                rec = smpool.tile([128, 1], F32, tag="rec")
                nc.vector.reciprocal(rec[:], tot[:])
                res_t = smpool.tile([128, ST_PER_B], F32, tag="res")
                nc.vector.tensor_scalar_mul(res_t[:], ex[:], rec[:])
                nc.sync.dma_start(
                    outd[bb].rearrange("(stl p) -> p stl", p=128), res_t[:])

            def emit_epilogue(st, e_ps):
                b_idx = st // ST_PER_B
                nc.vector.tensor_add(e_ps[:], e_ps[:], c2bc_sb[b_idx][:])
                relu_out = spool.tile([128, OUT], BF16, tag="relu")
                nc.vector.scalar_tensor_tensor(
                    relu_out[:], e_ps[:], 0.0, vbc[:],
                    op0=ALU.max, op1=ALU.mult,
                    accum_out=att_all[:, st:st + 1])
                if st % ST_PER_B == ST_PER_B - 1:
                    emit_softmax(st // ST_PER_B)

            for st in range(PHASE_TILES):
                emit_mm(e_tiles[st], 1, st)
                emit_epilogue(st, e_tiles[st])

            # --- steady state ----------------------------------------------
            next_block = 1
            for st in range(PHASE_TILES, N_ST):
                if st == 12:
                    emit_c2_bank(1, 0)
                    emit_c2_bank(1, 1)
                    emit_c2bc(1)
                e_ps = eps.tile([128, OUT], F32, tag="eps")
                emit_mm(e_ps, 0, st)
                emit_mm(e_ps, 1, st)
                emit_epilogue(st, e_ps)
                if next_block < N_SB and st + 3 >= NAT + next_block * 8 - 8:
                    emit_xblock(next_block, xts)
                    next_block += 1

    nc.compile()
    return nc


_NC = None


def _get_nc():
    global _NC
    if _NC is None:
        _NC = build_nc()
    return _NC


def _in_maps(hidden, encoder_outputs, c_t, W, b, v):
    hidden = np.ascontiguousarray(hidden, dtype=np.float32)
    encoder_outputs = np.ascontiguousarray(encoder_outputs, dtype=np.float32)
    c_t = np.ascontiguousarray(c_t, dtype=np.float32)
    W = np.ascontiguousarray(W, dtype=np.float32)
    b = np.ascontiguousarray(b, dtype=np.float32)
    v = np.ascontiguousarray(v, dtype=np.float32)
    maps = []
    for i in range(N_CORES):
        bs = slice(i * B_LOC, (i + 1) * B_LOC)
        maps.append({
            "hidden": hidden[bs].reshape(S_LOC, D),
            "enc": encoder_outputs[bs].reshape(S_LOC, D),
            "ct": c_t[bs],
            "W": W, "b": b, "v": v,
        })
    return maps


def run(hidden, encoder_outputs, c_t, W, b, v, trace=False, tmpdir=None):
    nc = _get_nc()
    maps = _in_maps(hidden, encoder_outputs, c_t, W, b, v)
    res = run_bass_kernel_spmd(nc, maps, list(range(N_CORES)), trace=trace,
                               tmpdir=tmpdir)
    out = np.concatenate([res.results[i]["out"] for i in range(N_CORES)],
                         axis=0)
    return out, res


def kernel(hidden, encoder_outputs, c_t, W, b, v):
    out, _ = run(hidden, encoder_outputs, c_t, W, b, v)
    return out


# revision 31
# speedup vs baseline: 1.1038x; 1.1038x over previous
"""Trainium2 Bass kernel for nn_Attention (sparse_attention variant).

Computes, for inputs hidden/encoder_outputs [B,S,D], c_t [B,D], W [OUT,3D],
b [OUT], v [OUT]:

    cat       = concat([hidden, broadcast(c_t), encoder_outputs], axis=2)
    energy    = relu(cat @ W.T + b)            # [B, S, OUT]
    attention = energy @ v                     # [B, S]
    out       = softmax(attention, axis=1)

Strategy (8 NeuronCores, data-parallel over batch, 2 batches/core):
  - Split W = [W1 | W2 | W3] over the feature axis.
  - Matmuls run in fp16 (fp32 PSUM accumulate).  The contraction dim f
    must sit on SBUF partitions, so X feeds in transposed.  The first
    4 s-tiles are loaded naturally (fp32, scalar ring, from t=0), DVE
    cast, and transposed on the PE while W streams in -- this lets the
    main loop start ~50us in instead of ~98us.  The remaining 28
    s-tiles go the proven scratch route: SWDGE DRAM->DRAM fp16 casts
    (gated on the first half of W so W wins the HBM race), then HWDGE
    xbar-transpose loads [f, s] on the sync ring.
  - W: fp32 half-row loads alternating sync/scalar rings, rows 0..511
    first; fp16-cast on DVE; transposed on the PE.  Bank-0 matmuls of
    s-tiles 0..2 plus the bank-0 half of c2 run as soon as W rows
    0..511 are on-chip (phase split), hiding the tail of the W load.
  - c2[b,:] = c_t[b] @ W2.T + b is computed per batch with [1,512]
    PSUM rows (batch 1 lazily at s-tile 12) and broadcast via a
    ones-column matmul.
  - Main loop per 128-row s-tile: accumulate pre[s, o] over 16 f-chunks
    x 2 PSUM banks; VectorE adds the broadcast c2 row and does a fused
    relu(pre)*v + row-sum (accum_out) -> attention logits.
  - Softmax over S=2048 per batch: 128x16 tile, DVE free-dim reduce +
    GpSimd partition all-reduce, ScalarE exp, DVE normalize.
"""

import sys
import numpy as np

for _p in ("/opt/trn_rl_repo",):
    if _p not in sys.path:
        sys.path.insert(0, _p)

import concourse.bass as bass
import concourse.bacc as bacc
import concourse.tile as tile
from concourse.tile import add_dep_helper
from concourse import mybir, bass_isa
from concourse.bass_utils import run_bass_kernel_spmd
from concourse.masks import make_identity

F32 = mybir.dt.float32
F16 = mybir.dt.float16
BF16 = mybir.dt.bfloat16
AF = mybir.ActivationFunctionType
ALU = mybir.AluOpType

B, S, D, OUT = 16, 2048, 1024, 1024
N_CORES = 8
B_LOC = B // N_CORES            # batches per core
S_LOC = B_LOC * S               # 4096 rows of X per core
N_ST = S_LOC // 128             # 32 s-tiles per core
ST_PER_B = S // 128             # 16 s-tiles per batch
FC = D // 128                   # 8 feature chunks per tensor
NB = OUT // 512                 # 2 PSUM banks across OUT
PHASE_TILES = 2                 # s-tiles that run bank0 before W fully loaded
NAT = 4                         # s-tiles via natural load + PE transpose
SB_ROWS = 1024                  # scratch s-block rows
N_SB = 4                        # scratch blocks cover st4..31


def build_nc():
    nc = bacc.Bacc("TRN2", target_bir_lowering=False, debug=False,
                   num_devices=N_CORES, dynamic_dma_scratch_size=32768)

    hid = nc.dram_tensor("hidden", [S_LOC, D], F32, kind="ExternalInput").ap()
    enc = nc.dram_tensor("enc", [S_LOC, D], F32, kind="ExternalInput").ap()
    ct = nc.dram_tensor("ct", [B_LOC, D], F32, kind="ExternalInput").ap()
    Wd = nc.dram_tensor("W", [OUT, 3 * D], F32, kind="ExternalInput").ap()
    bd = nc.dram_tensor("b", [OUT], F32, kind="ExternalInput").ap()
    vd = nc.dram_tensor("v", [OUT], F32, kind="ExternalInput").ap()
    outd = nc.dram_tensor("out", [B_LOC, S], F32, kind="ExternalOutput").ap()
    # scratch blocks: rows 512.., 1536.., 2560.., 3584.. (last = 512 rows)
    SBR = [1024, 1024, 1024, 512]
    SB0 = [512, 1536, 2560, 3584]
    scr_h = [nc.dram_tensor(f"scr_h{i}", [SBR[i], D], F16).ap()
             for i in range(N_SB)]
    scr_e = [nc.dram_tensor(f"scr_e{i}", [SBR[i], D], F16).ap()
             for i in range(N_SB)]

    with tile.TileContext(nc) as tc:
        with (
            tc.tile_pool(name="const", bufs=1) as cpool,
            tc.tile_pool(name="wT", bufs=1) as wpool,
            tc.tile_pool(name="wload", bufs=2) as wload,
            tc.tile_pool(name="xnat", bufs=1) as xnat,
            tc.tile_pool(name="xT", bufs=1) as xTp,
            tc.tile_pool(name="scratch", bufs=1) as spool,
            tc.tile_pool(name="sm", bufs=2) as smpool,
            tc.tile_pool(name="eps", bufs=2, space=bass.MemorySpace.PSUM) as eps,
            tc.tile_pool(name="cps", bufs=1, space=bass.MemorySpace.PSUM) as cps,
            tc.tile_pool(name="ptp", bufs=2, space=bass.MemorySpace.PSUM) as ptp,
        ):
            ident = cpool.tile([128, 128], F16)
            make_identity(nc, ident[:])
            ones_k1 = cpool.tile([1, 128], F16)
            nc.vector.memset(ones_k1[:], 1.0)
            att_all = cpool.tile([128, N_ST], F32)   # attention logits

            # ---- natural loads for the first NAT s-tiles (scalar ring) ----
            x32s = []
            for st in range(NAT):
                rows = slice(st * 128, (st + 1) * 128)
                x32h = xnat.tile([128, D], F32, tag="x32", bufs=2)
                nc.scalar.dma_start(x32h[:], hid[rows, :])
                x32e = xnat.tile([128, D], F32, tag="x32e", bufs=2)
                nc.scalar.dma_start(x32e[:], enc[rows, :])
                x32s.append((x32h, x32e))

            # ---- small constants ------------------------------------------
            v_h = cpool.tile([1, OUT], F16)
            nc.gpsimd.dma_start(v_h[:], vd[None, :])
            b_f = cpool.tile([1, OUT], F16)
            nc.gpsimd.dma_start(b_f[:], bd[None, :])
            ctT_f = cpool.tile([128, FC, B_LOC], F32)
            for bb in range(B_LOC):
                nc.sync.dma_start(ctT_f[:, :, bb],
                                  ct[bb].rearrange("(fc p) -> p fc", p=128))
            ctT_h = cpool.tile([128, FC, B_LOC], F16)
            nc.vector.tensor_copy(ctT_h[:], ctT_f[:])

            # ---- W: fp32 half-row loads, rows 0..511 first ----------------
            # wT[p, j, o] = W[o, j*128+p]; j in [0,8)=W1, [8,16)=W2,
            # [16,24)=W3.
            wT = wpool.tile([128, 3 * FC, OUT], F16)
            HALF_F = 3 * D // 2
            w_dmas = []
            w16s = {}

            def load_w(oc):
                w16 = []
                for half in range(2):
                    w_nat = wload.tile([128, HALF_F], F32, tag="wnat", bufs=2)
                    w_eng = nc.sync if (2 * oc + half) % 2 == 0 else nc.scalar
                    w_dma = w_eng.dma_start(
                        w_nat[:], Wd[oc * 128:(oc + 1) * 128,
                                     half * HALF_F:(half + 1) * HALF_F])
                    w_dmas.append(w_dma)
                    w_s = wload.tile([128, HALF_F], F16, tag="ws", bufs=4)
                    nc.vector.tensor_copy(w_s[:], w_nat[:])
                    w16.append(w_s)
                w16s[oc] = w16

            def pe_transpose_w(oc, jgs):
                for j0 in jgs:
                    half = j0 // 12
                    pt = ptp.tile([128, 4, 128], F16, tag="tp")
                    for k in range(4):
                        j = j0 + k - half * 12
                        nc.tensor.transpose(
                            pt[:, k, :],
                            w16s[oc][half][:, j * 128:(j + 1) * 128],
                            ident[:])
                    nc.vector.tensor_copy(
                        wT[:, j0:j0 + 4, oc * 128:(oc + 1) * 128], pt[:])

            for oc in range(4):
                load_w(oc)

            # natural-X casts + PE transposes for st0..NAT-1, interleaved
            # with the W transposes of rows 0..511
            def pe_transpose_x(st):
                x16h = xnat.tile([128, D], F16, tag="x16", bufs=1)
                nc.vector.tensor_copy(x16h[:], x32s[st][0][:])
                x16e = xnat.tile([128, D], F16, tag="x16e", bufs=1)
                nc.vector.tensor_copy(x16e[:], x32s[st][1][:])
                xTh = xTp.tile([128, FC, 128], F16, tag="xTn", bufs=4)
                xTe = xTp.tile([128, FC, 128], F16, tag="xTne", bufs=4)
                for src, dst in ((x16h, xTh), (x16e, xTe)):
                    for j0 in (0, 4):
                        pt = ptp.tile([128, 4, 128], F16, tag="tp")
                        for k in range(4):
                            j = j0 + k
                            nc.tensor.transpose(
                                pt[:, k, :], src[:, j * 128:(j + 1) * 128],
                                ident[:])
                        nc.vector.tensor_copy(dst[:, j0:j0 + 4, :], pt[:])
                return (xTh, xTe, 0)

            xts = {}
            for st in range(NAT):
                xts[st] = pe_transpose_x(st)
            for oc in range(4):
                pe_transpose_w(oc, (0, 4, 8, 12, 16, 20))

            # ---- X scratch casts, gated on the first half of W ------------
            gate = w_dmas[7]
            for i in range(N_SB):
                ch = nc.gpsimd.dma_start(
                    scr_h[i][:], hid[SB0[i]:SB0[i] + SBR[i], :])
                add_dep_helper(ch.ins, gate.ins,
                               reason="X casts yield HBM to W loads")
                ce = nc.gpsimd.dma_start(
                    scr_e[i][:], enc[SB0[i]:SB0[i] + SBR[i], :])
                add_dep_helper(ce.ins, gate.ins,
                               reason="X casts yield HBM to W loads")

            # rest of W
            for oc in range(4, 8):
                load_w(oc)

            # ---- scratch-block xbar loads (sync ring) ---------------------
            def emit_xblock(i, xts):
                xTh = xTp.tile([128, FC, SB_ROWS], F16, tag="xTh", bufs=2)
                xTe = xTp.tile([128, FC, SB_ROWS], F16, tag="xTe", bufs=2)
                r = SBR[i]
                for fc in range(FC):
                    nc.sync.dma_start(xTh[:, fc, :r],
                                      scr_h[i][:, fc * 128:(fc + 1) * 128],
                                      transpose=True)
                    nc.sync.dma_start(xTe[:, fc, :r],
                                      scr_e[i][:, fc * 128:(fc + 1) * 128],
                                      transpose=True)
                for k in range(r // 128):
                    xts[NAT + i * 8 + k] = (xTh, xTe, k * 128)

            def emit_mm(e_ps, ob, st):
                xTh, xTe, off = xts[st]
                ssl = slice(off, off + 128)
                sl = slice(ob * 512, (ob + 1) * 512)
                for fc in range(FC):
                    nc.tensor.matmul(e_ps[:, sl], xTh[:, fc, ssl],
                                     wT[:, fc, sl],
                                     start=(fc == 0), stop=False)
                for fc in range(FC):
                    nc.tensor.matmul(e_ps[:, sl], xTe[:, fc, ssl],
                                     wT[:, 2 * FC + fc, sl],
                                     start=False, stop=(fc == FC - 1))

            # vbc[p, o] = v[o] via PSUM half-tiles
            vbc = cpool.tile([128, OUT], F16)
            for ob in range(NB):
                sl = slice(ob * 512, (ob + 1) * 512)
                vps = cps.tile([128, 512], F32, tag="cps")
                nc.tensor.matmul(vps[:], ones_k1[:], v_h[:, sl],
                                 start=True, stop=True)
                nc.vector.tensor_copy(vbc[:, sl], vps[:])

            # c2[b, o] = c_t[b] @ W2.T + b via PSUM half-tiles
            c2b_sb = {}
            c2bc_sb = {}

            def emit_c2_bank(bb, ob):
                sl = slice(ob * 512, (ob + 1) * 512)
                cp = cps.tile([1, 512], F32, tag="cps")
                for fc in range(FC):
                    nc.tensor.matmul(cp[:], ctT_h[:, fc, bb:bb + 1],
                                     wT[:, FC + fc, sl],
                                     start=(fc == 0), stop=(fc == FC - 1))
                if bb not in c2b_sb:
                    c2b_t = cpool.tile([1, OUT], F16, tag=f"c2b_{bb}")
                    c2b_sb[bb] = c2b_t
                nc.vector.tensor_add(c2b_sb[bb][:, sl], cp[:], b_f[:, sl])

            def emit_c2bc(bb):
                if bb not in c2bc_sb:
                    c2bc_t = cpool.tile([128, OUT], F16, tag=f"c2bc_{bb}")
                    c2bc_sb[bb] = c2bc_t
                for ob in range(NB):
                    sl = slice(ob * 512, (ob + 1) * 512)
                    cp = cps.tile([128, 512], F32, tag="cps")
                    nc.tensor.matmul(cp[:], ones_k1[:], c2b_sb[bb][:, sl],
                                     start=True, stop=True)
                    nc.vector.tensor_copy(c2bc_sb[bb][:, sl], cp[:])

            # --- phase A: bank 0 only (W rows 0..511 suffice) --------------
            emit_c2_bank(0, 0)
            e_tiles = {}
            for st in range(PHASE_TILES):
                e_ps = eps.tile([128, OUT], F32, tag="eps")
                e_tiles[st] = e_ps
                emit_mm(e_ps, 0, st)

            # W transposes for rows 512..1023 (PE, overlapped with phase A);
            # half-0 j-groups first — their casts land before half-1's
            for oc in range(4, 8):
                pe_transpose_w(oc, (0, 4, 8))
            for oc in range(4, 8):
                pe_transpose_w(oc, (12, 16, 20))

            # --- phase B ---------------------------------------------------
            emit_c2_bank(0, 1)
            emit_c2bc(0)

            def emit_softmax(bb):
                sl = slice(bb * ST_PER_B, (bb + 1) * ST_PER_B)
                m1 = smpool.tile([128, 1], F32, tag="m1")
                nc.vector.tensor_reduce(m1[:], att_all[:, sl],
                                        axis=mybir.AxisListType.X,
                                        op=ALU.max)
                mall = smpool.tile([128, 1], F32, tag="mall")
                nc.gpsimd.partition_all_reduce(mall[:], m1[:], channels=128,
                                               reduce_op=bass_isa.ReduceOp.max)
                nmall = smpool.tile([128, 1], F32, tag="nmall")
                nc.vector.tensor_scalar_mul(nmall[:], mall[:], -1.0)
                ex = smpool.tile([128, ST_PER_B], F32, tag="ex")
                rs = smpool.tile([128, 1], F32, tag="rs")
                nc.scalar.activation(ex[:], att_all[:, sl], AF.Exp,
                                     bias=nmall[:], accum_out=rs[:])
                tot = smpool.tile([128, 1], F32, tag="tot")
                nc.gpsimd.partition_all_reduce(tot[:], rs[:], channels=128,
                                               reduce_op=bass_isa.ReduceOp.add)
                rec = smpool.tile([128, 1], F32, tag="rec")
                nc.vector.reciprocal(rec[:], tot[:])
                res_t = smpool.tile([128, ST_PER_B], F32, tag="res")
                nc.vector.tensor_scalar_mul(res_t[:], ex[:], rec[:])
                nc.sync.dma_start(
                    outd[bb].rearrange("(stl p) -> p stl", p=128), res_t[:])

            def emit_epilogue(st, e_ps):
                b_idx = st // ST_PER_B
                nc.vector.tensor_add(e_ps[:], e_ps[:], c2bc_sb[b_idx][:])
                relu_out = spool.tile([128, OUT], BF16, tag="relu")
                nc.vector.scalar_tensor_tensor(
                    relu_out[:], e_ps[:], 0.0, vbc[:],
                    op0=ALU.max, op1=ALU.mult,
                    accum_out=att_all[:, st:st + 1])
                if st % ST_PER_B == ST_PER_B - 1:
                    emit_softmax(st // ST_PER_B)

            for st in range(PHASE_TILES):
                emit_mm(e_tiles[st], 1, st)
                emit_epilogue(st, e_tiles[st])

            # first scratch block's loads
            emit_xblock(0, xts)

            # --- steady state ----------------------------------------------
            next_block = 1
            for st in range(PHASE_TILES, N_ST):
                if st == 12:
                    emit_c2_bank(1, 0)
                    emit_c2_bank(1, 1)
                    emit_c2bc(1)
                e_ps = eps.tile([128, OUT], F32, tag="eps")
                emit_mm(e_ps, 0, st)
                emit_mm(e_ps, 1, st)
                emit_epilogue(st, e_ps)
                if next_block < N_SB and st + 3 >= NAT + next_block * 8 - 8:
                    emit_xblock(next_block, xts)
                    next_block += 1

    nc.compile()
    return nc


_NC = None


def _get_nc():
    global _NC
    if _NC is None:
        _NC = build_nc()
    return _NC


def _in_maps(hidden, encoder_outputs, c_t, W, b, v):
    hidden = np.ascontiguousarray(hidden, dtype=np.float32)
    encoder_outputs = np.ascontiguousarray(encoder_outputs, dtype=np.float32)
    c_t = np.ascontiguousarray(c_t, dtype=np.float32)
    W = np.ascontiguousarray(W, dtype=np.float32)
    b = np.ascontiguousarray(b, dtype=np.float32)
    v = np.ascontiguousarray(v, dtype=np.float32)
    maps = []
    for i in range(N_CORES):
        bs = slice(i * B_LOC, (i + 1) * B_LOC)
        maps.append({
            "hidden": hidden[bs].reshape(S_LOC, D),
            "enc": encoder_outputs[bs].reshape(S_LOC, D),
            "ct": c_t[bs],
            "W": W, "b": b, "v": v,
        })
    return maps


def run(hidden, encoder_outputs, c_t, W, b, v, trace=False, tmpdir=None):
    nc = _get_nc()
    maps = _in_maps(hidden, encoder_outputs, c_t, W, b, v)
    res = run_bass_kernel_spmd(nc, maps, list(range(N_CORES)), trace=trace,
                               tmpdir=tmpdir)
    out = np.concatenate([res.results[i]["out"] for i in range(N_CORES)],
                         axis=0)
    return out, res


def kernel(hidden, encoder_outputs, c_t, W, b, v):
    out, _ = run(hidden, encoder_outputs, c_t, W, b, v)
    return out
